# revision 3
# baseline (speedup 1.0000x reference)
"""Matryoshka soft-top-k gating kernel for Trainium2 (Bass/Tile) — v2.1.

Computes, for each matryoshka scale k in (128, 64, 32):
    scores  = emb @ w  (+ b, which cancels in scores - threshold)
    srm     = (scores + C0) * mask      (order-preserving shift: masked
              positions sit at 0, far below every real score near C0)
    thr_k   = k-th largest srm per row (max8 + match_replace chains)
    gate    = sigmoid((srm - thr_k) * temperature)  (masked -> ~0)
    out_k   = emb * gate[..., None]

Sharding: data-parallel over the batch axis across 8 NeuronCores
(64 rows per core); w/temperature replicated, mask sharded with batch.

HBM traffic is the roofline (16 MiB/core at ~360 B/ns => ~47 us), so
embeddings upload as fp16 pre-scaled by 2^10 and outputs store as bf16;
the host pre-transposes embeddings into token-major [128, NT*D] and
reassembles/unscales the outputs.

Engine plan (78.3 us -> ~67 us in the TimelineSim cost model):
  - Gating runs as wide DVE tensor_tensor ops in 2x mode: the gate
    operand is materialized as bf16 PAIRS (each gate duplicated, AP
    innermost [1,2]) so every operand is 2-byte packed -> ~69 ns/tile
    instead of 1x broadcast TT (133), Pool TT (266) or ACT act-copy
    (330).  bf16 (not fp16) because near-zero gates underflow fp16's
    6e-8 floor, which shows up against the harness' 1e-6 rel floor.
    Pool and ACT co-gate four units (g0/g1 on Pool TT, g2/g3 as ACT
    per-tile activations with f32 scale columns).
  - A dedicated "mini" chain for rows 0-15 on its own partition-0
    tiles starts the moment chunks c0/c1 are scored, so store unit
    (32,0) is ready roughly when the input load drains.  Engine ops
    need 32-aligned partition starts, so 16-row slices of the shared
    srm are illegal — a separate tile sidesteps that.
  - One shared row-major score tile srm/work [64,256] with chains on
    32-aligned slices (AB rows 0-31, CC rows 32-63, F all): AB 1-8,
    CC 1-8, F 9-16 supply every remaining threshold.
  - The mask folds into the PSUM drain of the score transposes as
    srm = (s + C0)*mask (one scalar_tensor_tensor per half); the
    explicit dif = srm - thr stays a separate Pool op because folding
    thr into the sigmoid bias cancels catastrophically in ACT's
    scale/bias path.
  - ebT (transposed-chunk) drains split ACT (half A) + DVE (half B);
    GPSIMD cannot touch PSUM, so Pool stays out of the score pipeline
    and spends its time on gating instead.
  - Stores are emitted in producer order, 8/16-tile granularity, so the
    serialized DMA engines restart as soon as each group is gated.
"""

import numpy as np

import concourse.bacc as bacc
import concourse.bass as bass
import concourse.mybir as mybir
import concourse.tile as tile
from concourse.bass_utils import run_bass_kernel_spmd

N_CORES = 8
B, T, D = 512, 256, 128
R = B // N_CORES          # rows (documents) per core
KS = (128, 64, 32)
# Masked fill via order-preserving shift: srm = (s + C0)*mask.  Unmasked
# scores land near C0 (+-6e3), masked at 0 (far below), and the uniform
# +C0 shift cancels in s - thr inside the sigmoid bias.  C0 is small
# enough that f32 keeps ~0.008 absolute score precision (scores are
# pre-scaled by 2^10, so that is ~8e-6 in model units).
C0 = 1.0e5
REPL = -1.0               # match_replace sentinel (below masked 0)
NT = R * T // 128         # 128-token tiles per core (128)
CH = 16                   # tiles per DMA chunk
NCH = NT // CH            # 8 chunks
UNIT_TILES = 32           # tiles per store unit (1 MiB)
N_UNITS = 4

EMB_DT = mybir.dt.float16
EMB_NP = np.float16
EMB_SCALE = 1024.0

f32 = mybir.dt.float32
f16 = mybir.dt.float16
bf16 = mybir.dt.bfloat16
u8 = mybir.dt.uint8
Alu = mybir.AluOpType
Act = mybir.ActivationFunctionType

# row ranges (engine partition starts must be 32-aligned; the 16-row
# A/B ranges live on their own partition-0-based mini tiles)
RANGES = {"A": (0, 16), "B": (16, 32), "AB": (0, 32), "CC": (32, 64),
          "F": (0, 64)}
# unit -> range owning its rows
UNIT_RANGE = {0: "A", 1: "B", 2: "CC", 3: "CC"}
# ebT split: partner engine takes the leading cols (ready after the
# first transposes), ACT the tail.  c0-c3 partner with DVE (fills DVE's
# idle front-end), c4-c7 with Pool.
EBT_SPLIT = {"dve": 768, "pool": 768}

PHASES = []


class _St:
    pass


def _bcast(ap, n):
    return bass.AP(tensor=ap.tensor, offset=ap.offset, ap=[[0, n]] + list(ap.ap))


def build_bass():
    nc = bacc.Bacc("TRN2", target_bir_lowering=False, debug=False)

    emb = nc.dram_tensor("emb_tm", [128, NT * D], EMB_DT, kind="ExternalInput")
    w = nc.dram_tensor("w", [D], f32, kind="ExternalInput")
    temp = nc.dram_tensor("temperature", [1], f32, kind="ExternalInput")
    mask = nc.dram_tensor("mask", [R, T], u8, kind="ExternalInput")
    out = nc.dram_tensor("out", [len(KS), 128, NT * D], bf16,
                         kind="ExternalOutput")
    ident_d = nc.inline_tensor(np.eye(128, dtype=np.float32), name="ident128")

    st = _St()
    st.nc = nc
    st.out = out

    with tile.TileContext(nc) as tc:
        with (
            tc.tile_pool(name="singles", bufs=1) as singles,
            tc.tile_pool(name="out0", bufs=8) as opool,
            tc.tile_pool(name="ebt", bufs=3) as ebtpool,
            tc.tile_pool(name="psum", bufs=2, space="PSUM") as psum,
        ):
            st.singles = singles
            st.opool = opool
            st.ebtpool = ebtpool
            st.psum = psum
            st.tc = tc
            _emit(st, nc, emb, w, temp, mask, ident_d)

    nc.compile()
    return nc


def _mark(st, label):
    PHASES.append((label, st.nc.next_id()))


# ---------------------------------------------------------------- scoring

def _score_transpose(st, ci, pos):
    """PE-transpose the chunk in two 8-tile halves, each to its own
    1-bank PSUM tile (bufs=4), drained by alternating engines so the
    transpose->SBUF pipeline keeps up with the load cadence."""
    _mark(st, f"scoreT_c{ci}")
    nc = st.nc
    ebT = st.ebtpool.tile([128, CH * 128], EMB_DT, tag="ebT")
    H = CH * 128 // 2
    for h in range(2):
        ph = st.psum.tile([128, H], EMB_DT, tag="ptile",
                          name=f"pt{ci}_{h}", bufs=4)
        for j in range(8):
            t = ci * CH + h * 8 + j
            nc.tensor.transpose(ph[:, j * 128:(j + 1) * 128],
                                st.embbuf[:, t * D:(t + 1) * D], st.ident16)
        dst = ebT[:, h * H:(h + 1) * H]
        # GPSIMD cannot read PSUM: drains go ACT (half A) + DVE (half B)
        if h == 0:
            nc.scalar.copy(dst, ph)
        else:
            nc.vector.tensor_copy(dst, ph)
    st.ebts[ci] = ebT


def _score_matvec(st, ci):
    """Matvec a transposed chunk against w_col; copy score columns out."""
    _mark(st, f"scoreM_c{ci}")
    nc = st.nc
    ebT = st.ebts[ci]
    psc = st.psum.tile([128, CH], f32, tag="psc", name=f"psc{ci}", bufs=2)
    for j in range(CH):
        nc.tensor.matmul(psc[:, j:j + 1], ebT[:, j * 128:(j + 1) * 128],
                         st.w_col)
    nc.vector.tensor_copy(st.scores_tm[:, ci * CH:(ci + 1) * CH], psc)


def _score_chunk(st, pos):
    """Process the pos-th chunk in CHUNK_ORDER; matvec lags by MV_LAG."""
    _score_transpose(st, CHUNK_ORDER[pos], pos)
    if pos >= MV_LAG:
        _score_matvec(st, CHUNK_ORDER[pos - MV_LAG])
    if pos == NCH - 1:
        for p in range(NCH - MV_LAG, NCH):
            _score_matvec(st, CHUNK_ORDER[p])


# ------------------------------------------------------------- thresholds

def _prefix(st, rng_name, tr_hi):
    """Build masked row-major scores srm[lo:hi]: PE transposes the
    even/odd tile columns straight out of scores_tm (strided read), and
    the PSUM drain folds the mask via srm = (s + C0)*mask in one
    scalar_tensor_tensor per half."""
    _mark(st, f"prefix_{rng_name}")
    nc = st.nc
    lo, hi = RANGES[rng_name]
    w_lo = (lo // 32) * 32
    # deinterleave even/odd tile columns (DVE strided copies), then PE
    # transpose and fold the mask in the PSUM drain
    n = hi - w_lo
    src = st.scores_tm
    for h, name in ((0, "e"), (1, "o")):
        sh = bass.AP(tensor=src.tensor, offset=src.offset + 2 * w_lo + h,
                     ap=[list(src.ap[0]), [2, n]])
        de = st.singles.tile([128, R], f32, tag=f"de{h}", name=f"de{h}")
        nc.vector.tensor_copy(de[:, w_lo:hi], sh)
        ps = st.psum.tile([n, 128], f32, tag=PFX_TAG,
                          name=f"ps{name}{rng_name}",
                          bufs=4 if PFX_TAG == "ptile" else 2)
        nc.tensor.transpose(ps, de[:, w_lo:hi], st.ident)
        nc.vector.scalar_tensor_tensor(
            out=st.srm[w_lo:hi, h * 128:(h + 1) * 128], in0=ps[0:n, :],
            scalar=C0, in1=st.mfs[w_lo:hi, h * 128:(h + 1) * 128],
            op0=Alu.add, op1=Alu.mult,
        )


ROUNDS_PRIO = False
C_SPLIT = False
CHUNK_ORDER = [0, 1, 2, 3, 4, 5, 6, 7]
PFX_TAG = "psc"
MV_LAG = 2
CHAIN_MODE = "merged"
EBT67_3WAY = False


def _rounds(st, rng_name, upto):
    """max8 + match_replace rounds on srm/work partition slice."""
    if ROUNDS_PRIO:
        with st.tc.high_priority():
            return _rounds_inner(st, rng_name, upto)
    return _rounds_inner(st, rng_name, upto)


def _rounds_inner(st, rng_name, upto):
    _mark(st, f"rounds_{rng_name}_{upto}")
    nc = st.nc
    lo, hi = RANGES[rng_name]
    rg = hi - lo
    done = st.rounds_done[lo:hi]
    assert (done == done[0]).all(), (rng_name, done)
    r0 = int(done[0])
    for r in range(r0, upto):
        if r not in st.mxr:
            st.mxr[r] = st.singles.tile([R, 8], f32, tag=f"mx_{r}",
                                        name=f"mx_{r}")
        mx = st.mxr[r][lo:hi, :]
        src = st.srm[lo:hi, :] if r == 0 else st.work[lo:hi, :]
        nc.vector.max(out=mx, in_=src)
        if r < 15:
            nc.vector.match_replace(out=st.work[lo:hi, :], in_to_replace=mx,
                                    in_values=src, imm_value=REPL)
    st.rounds_done[lo:hi] = upto
    return st.mxr[upto - 1][lo:hi, 7:8]


# ------------------------------------------------------------------ gates

GCOLS_PRIO = False


def _gcols(st, rng_name, k, thr, pair_eng="act", act_cols=False):
    """thr -> bias col -> sigmoid (f16) -> PE transpose -> paired gate
    cols gpair[(k, rng)] [128, 4*rg] (col 4j+2h+b for row lo+j, half h).
    act_cols additionally emits f32 columns (tile-local index) for
    ACT-assisted gating (Activation scale must be FP32)."""
    if GCOLS_PRIO:
        with st.tc.high_priority():
            _gcols_inner(st, rng_name, k, thr, pair_eng, act_cols)
    else:
        _gcols_inner(st, rng_name, k, thr, pair_eng, act_cols)


def _gcols_inner(st, rng_name, k, thr, pair_eng, act_cols=False):
    _mark(st, f"gcols_{rng_name}_{k}")
    nc = st.nc
    lo, hi = RANGES[rng_name]
    rg = hi - lo
    # dif = srm - thr on Pool (avoids the huge-offset cancellation the
    # folded bias would hit in ACT's scale/bias pipeline)
    if k not in st.dif:
        st.dif[k] = st.singles.tile([R, T], f32, tag=f"dif{k}",
                                    name=f"dif{k}")
    dif = st.dif[k][lo:hi, :]
    nc.gpsimd.tensor_scalar(out=dif, in0=st.srm[lo:hi, :], scalar1=thr,
                            scalar2=None, op0=Alu.subtract)
    if k not in st.sig:
        st.sig[k] = st.singles.tile([R, T], bf16, tag=f"sig{k}",
                                    name=f"sig{k}")
    sig = st.sig[k][lo:hi, :]
    nc.scalar.activation(sig, dif, Act.Sigmoid, bias=0.0,
                         scale=st.temp_col[lo:hi, :])
    g = st.singles.tile([128, 4 * rg], bf16, tag=f"g{rng_name}_{k}",
                        name=f"g{rng_name}_{k}")
    st.gpair[(k, rng_name)] = g
    g32 = None
    if act_cols:
        g32 = st.singles.tile([128, 2 * rg], f32, tag=f"g32{rng_name}_{k}",
                              name=f"g32{rng_name}_{k}")
        st.g32[(k, rng_name)] = g32
    cp = nc.scalar.copy if pair_eng == "act" else nc.vector.tensor_copy
    # PE needs 0/32/64-aligned base partitions: transpose a 32-aligned
    # window of the full-height sig tile, then slice the wanted rows.
    w_lo = 0 if lo < 32 else 32
    w_hi = 64 if hi > w_lo + 32 or rg > 32 else w_lo + 32
    w = w_hi - w_lo
    for h in range(2):
        pm = st.psum.tile([128, w], bf16, tag="pst",
                          name=f"pm{rng_name}{k}{h}", bufs=2)
        nc.tensor.transpose(pm, st.sig[k][w_lo:w_hi, h * 128:(h + 1) * 128],
                            st.identb[w_lo:w_hi, w_lo:w_hi])
        dst = bass.AP(tensor=g.tensor, offset=g.offset + 2 * h,
                      ap=[list(g.ap[0]), [4, rg], [1, 2]])
        src = bass.AP(tensor=pm.tensor, offset=pm.offset + (lo - w_lo),
                      ap=[list(pm.ap[0]), [1, rg], [0, 2]])
        cp(dst, src)
        if act_cols:
            d32 = bass.AP(tensor=g32.tensor, offset=g32.offset + h,
                          ap=[list(g32.ap[0]), [2, rg]])
            s32 = bass.AP(tensor=pm.tensor, offset=pm.offset + (lo - w_lo),
                          ap=[list(pm.ap[0]), [1, rg]])
            nc.scalar.copy(d32, s32)


def _gate(st, k, unit, eng, g_lo=0, n_g=4, store=True):
    """Gate groups [g_lo, g_lo+n_g) (8 tiles each) of unit on an engine.

    eng: 'dve' (one wide 2x TT), 'pool' (one TT per group), 'act'
    (per-tile activation copies).  store=True emits the group stores
    immediately after."""
    _mark(st, f"gate_{k}_{unit}_{eng}{g_lo}")
    nc = st.nc
    cand = ("F", "AB" if unit < 2 else "CC", UNIT_RANGE[unit])
    for rng_name in cand:
        if (k, rng_name) in st.gpair:
            break
    lo, hi = RANGES[rng_name]
    g = st.gpair[(k, rng_name)]
    och = st.och[(k, unit)]
    t0 = unit * UNIT_TILES
    if eng == "dve":
        # two-group (16-tile) ops: small enough to fill chain holes
        gis = range(g_lo, g_lo + n_g, 2)
    else:
        gis = range(g_lo, g_lo + n_g)
    for gi in gis:
        ts = t0 + gi * 8
        n = 16 if eng == "dve" else 8
        col0 = 2 * (ts - 2 * lo)
        gb = bass.AP(tensor=g.tensor, offset=g.offset + col0,
                     ap=[list(g.ap[0]), [2, n], [0, 64], [1, 2]])
        o_ap = och[:, (ts - t0) * D:(ts - t0 + n) * D]
        e_ap = st.embbuf[:, ts * D:(ts + n) * D]
        if eng == "dve":
            nc.vector.tensor_tensor(o_ap, e_ap, gb, op=Alu.mult)
        elif eng == "pool":
            nc.gpsimd.tensor_tensor(o_ap, e_ap, gb, op=Alu.mult)
        else:  # act: per-tile activation with f32 scale col
            g32 = st.g32[(k, rng_name)]
            for j in range(8):
                t = ts + j
                scol = g32[:, (t - 2 * lo):(t - 2 * lo) + 1]
                nc.scalar.activation(
                    och[:, (t - t0) * D:(t - t0 + 1) * D],
                    st.embbuf[:, t * D:(t + 1) * D], Act.Copy,
                    bias=0.0, scale=scol)
        if store:
            _store(st, k, unit, gi, 2 if eng == "dve" else 1)


def _store(st, k, unit, g_lo=0, n_g=4):
    _mark(st, f"store_{k}_{unit}_{g_lo}")
    nc = st.nc
    k_i = KS.index(k)
    t0 = unit * UNIT_TILES + g_lo * 8
    n = n_g * 8
    och = st.och[(k, unit)]
    nc.sync.dma_start(
        out=st.out.ap()[k_i, :, t0 * D:(t0 + n) * D],
        in_=och[:, (t0 - unit * UNIT_TILES) * D:
                (t0 - unit * UNIT_TILES + n) * D],
    )


def _new_och(st, k, unit):
    st.och[(k, unit)] = st.opool.tile([128, UNIT_TILES * D], bf16, tag="och",
                                      name=f"och{k}_{unit}")


def _mini(st, mname, row_lo):
    """Early k=32 chain for 16 rows on dedicated partition-0 tiles.

    Rows [row_lo, row_lo+16) can't be partition-sliced out of the shared
    srm (engine ops need 32-aligned starts), but a dedicated tile starts
    at partition 0, so the chain can run the moment its two chunks are
    scored — unit (32, row_lo//16) stores while later chunks still load."""
    _mark(st, f"mini_{mname}")
    nc = st.nc
    rg = 16
    srmx = st.singles.tile([rg, T], f32, tag=f"srm{mname}", name=f"srm{mname}")
    workx = st.singles.tile([rg, T], f32, tag=f"work{mname}",
                            name=f"work{mname}")
    mfx = st.mfs_mini[mname]
    src = st.scores_tm
    for h in range(2):
        sh = bass.AP(tensor=src.tensor, offset=src.offset + 2 * row_lo + h,
                     ap=[list(src.ap[0]), [2, rg]])
        de = st.singles.tile([128, rg], f32, tag=f"dem{mname}{h}",
                             name=f"dem{mname}{h}")
        nc.vector.tensor_copy(de, sh)
        ps = st.psum.tile([rg, 128], f32, tag="psc", name=f"psm{mname}{h}",
                          bufs=2)
        nc.tensor.transpose(ps, de, st.ident)
        nc.vector.scalar_tensor_tensor(
            out=srmx[:, h * 128:(h + 1) * 128], in0=ps,
            scalar=C0, in1=mfx[:, h * 128:(h + 1) * 128],
            op0=Alu.add, op1=Alu.mult,
        )
    for r in range(4):
        mx = st.singles.tile([rg, 8], f32, tag=f"mxm{mname}{r}",
                             name=f"mxm{mname}{r}")
        s0 = srmx if r == 0 else workx
        nc.vector.max(out=mx, in_=s0)
        nc.vector.match_replace(out=workx, in_to_replace=mx, in_values=s0,
                                imm_value=REPL)
        thr = mx[:, 7:8]
    dif = st.singles.tile([rg, T], f32, tag=f"difm{mname}",
                          name=f"difm{mname}")
    nc.gpsimd.tensor_scalar(out=dif, in0=srmx, scalar1=thr, scalar2=None,
                            op0=Alu.subtract)
    sig = st.singles.tile([rg, T], bf16, tag=f"sigm{mname}",
                          name=f"sigm{mname}")
    nc.scalar.activation(sig, dif, Act.Sigmoid, bias=0.0,
                         scale=st.temp_col[0:rg, :])
    g = st.singles.tile([128, 4 * rg], bf16, tag=f"gm{mname}",
                        name=f"gm{mname}")
    st.gpair[(32, mname)] = g
    for h in range(2):
        pm = st.psum.tile([128, rg], bf16, tag="pst", name=f"pmm{mname}{h}",
                          bufs=2)
        nc.tensor.transpose(pm, sig[:, h * 128:(h + 1) * 128],
                            st.identb[0:rg, 0:rg])
        dst = bass.AP(tensor=g.tensor, offset=g.offset + 2 * h,
                      ap=[list(g.ap[0]), [4, rg], [1, 2]])
        s2 = bass.AP(tensor=pm.tensor, offset=pm.offset,
                     ap=[list(pm.ap[0]), [1, rg], [0, 2]])
        nc.vector.tensor_copy(dst, s2)


# ------------------------------------------------------------------- emit

def _emit(st, nc, emb, w, temp, mask, ident_d):
    singles = st.singles
    st.ebts = {}
    st.mxr = {}
    st.sig = {}
    st.dif = {}
    st.g32 = {}
    st.gpair = {}
    st.och = {}
    st.rounds_done = np.zeros(R, dtype=int)

    # ---- loads (sync queue order == service order) ----
    embbuf = singles.tile([128, NT * D], EMB_DT)
    st.embbuf = embbuf

    def load_chunk(ci):
        nc.sync.dma_start(
            out=embbuf[:, ci * CH * D:(ci + 1) * CH * D],
            in_=emb.ap()[:, ci * CH * D:(ci + 1) * CH * D])

    # PE p-state warmup on a memset tile (runs before any DMA lands)
    wtile = singles.tile([128, 128], f32)
    nc.gpsimd.memset(wtile, 0.0)
    pwarm = st.psum.tile([128, 128], f32, tag="pst", name="pwarm", bufs=2)
    for _ in range(10):
        nc.tensor.transpose(pwarm, wtile, wtile)

    ident = singles.tile([128, 128], f32)
    nc.sync.dma_start(out=ident, in_=ident_d.ap())
    st.ident = ident
    load_chunk(CHUNK_ORDER[0])
    load_chunk(CHUNK_ORDER[1])
    w_col_stage = singles.tile([128, 1], f32)
    nc.sync.dma_start(
        out=w_col_stage,
        in_=bass.AP(tensor=w.ap().tensor, offset=0, ap=[[1, 128], [0, 1]]))
    mask_sb = singles.tile([R, T], u8)
    nc.sync.dma_start(out=mask_sb, in_=mask.ap())
    st.mask_sb = mask_sb
    mask_b = singles.tile([16, T], u8)
    nc.sync.dma_start(out=mask_b, in_=mask.ap()[16:32, :])
    st.mask_b = mask_b
    tc_stage = singles.tile([R, 1], f32)
    nc.sync.dma_start(out=tc_stage, in_=_bcast(temp.ap(), R))
    for p in range(2, NCH):
        load_chunk(CHUNK_ORDER[p])

    # ---- SBUF constants (DVE: keep ACT/Pool queues free for ebT) ----
    sig_warm = singles.tile([1, 1], f32)
    nc.scalar.activation(sig_warm, ident[0:1, 0:1], Act.Sigmoid,
                         bias=0.0, scale=1.0)
    ident16 = singles.tile([128, 128], EMB_DT)
    nc.vector.tensor_copy(ident16, ident)
    st.ident16 = ident16
    identb = singles.tile([128, 128], bf16)
    nc.vector.tensor_copy(identb, ident)
    st.identb = identb
    w_col = singles.tile([128, 1], EMB_DT)
    nc.vector.tensor_copy(w_col, w_col_stage)
    st.w_col = w_col
    temp_col = singles.tile([R, 1], f32)
    nc.vector.tensor_scalar_mul(temp_col, tc_stage, 1.0 / EMB_SCALE)
    st.temp_col = temp_col
    negtemp_col = singles.tile([R, 1], f32)
    nc.vector.tensor_scalar_mul(negtemp_col, tc_stage, -1.0 / EMB_SCALE)
    st.negtemp_col = negtemp_col

    st.scores_tm = singles.tile([128, NT], f32)
    nc.vector.memset(st.scores_tm, 0.0)
    st.srm = singles.tile([R, T], f32)
    st.work = singles.tile([R, T], f32)
    mfs = singles.tile([R, T], f32)
    nc.vector.tensor_copy(mfs, mask_sb)
    st.mfs = mfs
    mfs_b = singles.tile([16, T], f32)
    nc.vector.tensor_copy(mfs_b, mask_b)
    st.mfs_mini = {"A": mfs[0:16, :], "B": mfs_b}

    # ---- pipeline ----
    _score_transpose(st, 0, 0)
    _score_transpose(st, 1, 1)
    _score_matvec(st, 0)
    _score_matvec(st, 1)
    _mini(st, "A", 0)
    _new_och(st, 32, 0)
    _gate(st, 32, 0, "dve")
    _score_transpose(st, 2, 2)
    _score_transpose(st, 3, 3)
    _score_matvec(st, 2)
    _score_matvec(st, 3)
    _prefix(st, "AB", 32)
    thrAB32 = _rounds(st, "AB", 4)
    _gcols(st, "AB", 32, thrAB32, pair_eng="dve", act_cols=True)
    _new_och(st, 32, 1)
    _gate(st, 32, 1, "pool", g_lo=0, n_g=2, store=True)
    _gate(st, 32, 1, "act", g_lo=2, n_g=2, store=True)
    _score_transpose(st, 4, 4)
    _score_transpose(st, 5, 5)
    _score_matvec(st, 4)
    _score_matvec(st, 5)
    _score_transpose(st, 6, 6)
    _score_transpose(st, 7, 7)
    _score_matvec(st, 6)
    _score_matvec(st, 7)
    _prefix(st, "CC", 64)
    thrC32 = _rounds(st, "CC", 4)
    _gcols(st, "CC", 32, thrC32)
    _new_och(st, 32, 2)
    _gate(st, 32, 2, "dve")
    _new_och(st, 32, 3)
    _gate(st, 32, 3, "dve")
    thrAB64 = _rounds(st, "AB", 8)
    _gcols(st, "AB", 64, thrAB64, act_cols=True)
    _new_och(st, 64, 0)
    _gate(st, 64, 0, "dve")
    # (64,1): Pool g0,g1 + ACT g2,g3 while DVE runs the CC chain
    _new_och(st, 64, 1)
    _gate(st, 64, 1, "pool", g_lo=0, n_g=2, store=True)
    _gate(st, 64, 1, "act", g_lo=2, n_g=2, store=True)
    thrC64 = _rounds(st, "CC", 8)
    _gcols(st, "CC", 64, thrC64, act_cols=True)
    _new_och(st, 64, 2)
    _gate(st, 64, 2, "pool", g_lo=0, n_g=2, store=True)
    _gate(st, 64, 2, "act", g_lo=2, n_g=2, store=True)
    _new_och(st, 64, 3)
    _gate(st, 64, 3, "dve")
    thr128 = _rounds(st, "F", 16)
    _gcols(st, "F", 128, thr128, act_cols=True)
    _new_och(st, 128, 0)
    _gate(st, 128, 0, "dve")
    _new_och(st, 128, 1)
    _gate(st, 128, 1, "pool", g_lo=0, n_g=2, store=True)
    _gate(st, 128, 1, "act", g_lo=2, n_g=2, store=True)
    _new_och(st, 128, 2)
    _gate(st, 128, 2, "dve")
    _new_och(st, 128, 3)
    _gate(st, 128, 3, "dve")


# -------------------------------------------------------------- host glue

_NC = None


def _get_nc():
    global _NC
    if _NC is None:
        _NC = build_bass()
    return _NC


def make_in_maps(embeddings, w, temperature, mask):
    emb = np.asarray(embeddings, dtype=np.float32)
    w = np.ascontiguousarray(np.asarray(w, dtype=np.float32))
    temp = np.ascontiguousarray(np.asarray(temperature, dtype=np.float32))
    mask_u8 = np.asarray(mask).astype(np.uint8)
    in_maps = []
    for c in range(N_CORES):
        sl = slice(c * R, (c + 1) * R)
        esh = emb[sl].reshape(NT, 128, D).transpose(1, 0, 2).reshape(128, NT * D)
        esh = esh * EMB_SCALE
        in_maps.append({
            "emb_tm": np.ascontiguousarray(esh.astype(EMB_NP)),
            "w": w,
            "temperature": temp,
            "mask": np.ascontiguousarray(mask_u8[sl]),
        })
    return in_maps


def postprocess(results):
    outs = []
    for r in results:
        o = np.asarray(r["out"]).astype(np.float32)
        o *= 1.0 / EMB_SCALE
        o = o.reshape(len(KS), 128, NT, D).transpose(0, 2, 1, 3)
        outs.append(o.reshape(len(KS), R, T, D))
    return np.concatenate(outs, axis=1)


def kernel(embeddings, w, b, temperature, mask):
    nc = _get_nc()
    in_maps = make_in_maps(embeddings, w, temperature, mask)
    res = run_bass_kernel_spmd(nc, in_maps, core_ids=list(range(N_CORES)))
    return postprocess(res.results)


# revision 4
# speedup vs baseline: 1.0109x; 1.0109x over previous
"""Matryoshka soft-top-k gating kernel for Trainium2 (Bass/Tile) — v2.1.

Computes, for each matryoshka scale k in (128, 64, 32):
    scores  = emb @ w  (+ b, which cancels in scores - threshold)
    srm     = (scores + C0) * mask      (order-preserving shift: masked
              positions sit at 0, far below every real score near C0)
    thr_k   = k-th largest srm per row (max8 + match_replace chains)
    gate    = sigmoid((srm - thr_k) * temperature)  (masked -> ~0)
    out_k   = emb * gate[..., None]

Sharding: data-parallel over the batch axis across 8 NeuronCores
(64 rows per core); w/temperature replicated, mask sharded with batch.

HBM traffic is the roofline (16 MiB/core at ~360 B/ns => ~47 us), so
embeddings upload as fp16 pre-scaled by 2^10 and outputs store as bf16;
the host pre-transposes embeddings into token-major [128, NT*D] and
reassembles/unscales the outputs.

Engine plan (78.3 us -> ~67 us in the TimelineSim cost model):
  - Gating runs as wide DVE tensor_tensor ops in 2x mode: the gate
    operand is materialized as bf16 PAIRS (each gate duplicated, AP
    innermost [1,2]) so every operand is 2-byte packed -> ~69 ns/tile
    instead of 1x broadcast TT (133), Pool TT (266) or ACT act-copy
    (330).  bf16 (not fp16) because near-zero gates underflow fp16's
    6e-8 floor, which shows up against the harness' 1e-6 rel floor.
    Pool and ACT co-gate four units (g0/g1 on Pool TT, g2/g3 as ACT
    per-tile activations with f32 scale columns).
  - A dedicated "mini" chain for rows 0-15 on its own partition-0
    tiles starts the moment chunks c0/c1 are scored, so store unit
    (32,0) is ready roughly when the input load drains.  Engine ops
    need 32-aligned partition starts, so 16-row slices of the shared
    srm are illegal — a separate tile sidesteps that.
  - One shared row-major score tile srm/work [64,256] with chains on
    32-aligned slices (AB rows 0-31, CC rows 32-63, F all): AB 1-8,
    CC 1-8, F 9-16 supply every remaining threshold.
  - The mask folds into the PSUM drain of the score transposes as
    srm = (s + C0)*mask (one scalar_tensor_tensor per half); the
    explicit dif = srm - thr stays a separate Pool op because folding
    thr into the sigmoid bias cancels catastrophically in ACT's
    scale/bias path.
  - ebT (transposed-chunk) drains split ACT (half A) + DVE (half B),
    with c2-c4 fully on ACT so the mini/AB chain rounds stay dense;
    GPSIMD cannot touch PSUM, so Pool stays out of the score pipeline
    and spends its time on gating instead.
  - Stores are emitted in producer order, 8/16-tile granularity, so the
    serialized DMA engines restart as soon as each group is gated.
"""

import numpy as np

import concourse.bacc as bacc
import concourse.bass as bass
import concourse.mybir as mybir
import concourse.tile as tile
from concourse.bass_utils import run_bass_kernel_spmd

N_CORES = 8
B, T, D = 512, 256, 128
R = B // N_CORES          # rows (documents) per core
KS = (128, 64, 32)
# Masked fill via order-preserving shift: srm = (s + C0)*mask.  Unmasked
# scores land near C0 (+-6e3), masked at 0 (far below), and the uniform
# +C0 shift cancels in s - thr inside the sigmoid bias.  C0 is small
# enough that f32 keeps ~0.008 absolute score precision (scores are
# pre-scaled by 2^10, so that is ~8e-6 in model units).
C0 = 1.0e5
REPL = -1.0               # match_replace sentinel (below masked 0)
NT = R * T // 128         # 128-token tiles per core (128)
CH = 16                   # tiles per DMA chunk
NCH = NT // CH            # 8 chunks
UNIT_TILES = 32           # tiles per store unit (1 MiB)
N_UNITS = 4

EMB_DT = mybir.dt.float16
EMB_NP = np.float16
EMB_SCALE = 1024.0

f32 = mybir.dt.float32
f16 = mybir.dt.float16
bf16 = mybir.dt.bfloat16
u8 = mybir.dt.uint8
Alu = mybir.AluOpType
Act = mybir.ActivationFunctionType

# row ranges (engine partition starts must be 32-aligned; the 16-row
# A/B ranges live on their own partition-0-based mini tiles)
RANGES = {"A": (0, 16), "B": (16, 32), "AB": (0, 32), "CC": (32, 64),
          "F": (0, 64)}
# unit -> range owning its rows
UNIT_RANGE = {0: "A", 1: "B", 2: "CC", 3: "CC"}
# ebT split: partner engine takes the leading cols (ready after the
# first transposes), ACT the tail.  c0-c3 partner with DVE (fills DVE's
# idle front-end), c4-c7 with Pool.
EBT_SPLIT = {"dve": 768, "pool": 768}

PHASES = []


class _St:
    pass


def _bcast(ap, n):
    return bass.AP(tensor=ap.tensor, offset=ap.offset, ap=[[0, n]] + list(ap.ap))


def build_bass():
    nc = bacc.Bacc("TRN2", target_bir_lowering=False, debug=False)

    emb = nc.dram_tensor("emb_tm", [128, NT * D], EMB_DT, kind="ExternalInput")
    w = nc.dram_tensor("w", [D], f32, kind="ExternalInput")
    temp = nc.dram_tensor("temperature", [1], f32, kind="ExternalInput")
    mask = nc.dram_tensor("mask", [R, T], u8, kind="ExternalInput")
    out = nc.dram_tensor("out", [len(KS), 128, NT * D], bf16,
                         kind="ExternalOutput")
    ident_d = nc.inline_tensor(np.eye(128, dtype=np.float32), name="ident128")

    st = _St()
    st.nc = nc
    st.out = out

    with tile.TileContext(nc) as tc:
        with (
            tc.tile_pool(name="singles", bufs=1) as singles,
            tc.tile_pool(name="out0", bufs=8) as opool,
            tc.tile_pool(name="ebt", bufs=3) as ebtpool,
            tc.tile_pool(name="psum", bufs=2, space="PSUM") as psum,
        ):
            st.singles = singles
            st.opool = opool
            st.ebtpool = ebtpool
            st.psum = psum
            st.tc = tc
            _emit(st, nc, emb, w, temp, mask, ident_d)

    nc.compile()
    return nc


def _mark(st, label):
    PHASES.append((label, st.nc.next_id()))


# ---------------------------------------------------------------- scoring

def _score_transpose(st, ci, pos):
    """PE-transpose the chunk in two 8-tile halves, each to its own
    1-bank PSUM tile (bufs=4), drained by alternating engines so the
    transpose->SBUF pipeline keeps up with the load cadence."""
    _mark(st, f"scoreT_c{ci}")
    nc = st.nc
    ebT = st.ebtpool.tile([128, CH * 128], EMB_DT, tag="ebT")
    H = CH * 128 // 2
    for h in range(2):
        ph = st.psum.tile([128, H], EMB_DT, tag="ptile",
                          name=f"pt{ci}_{h}", bufs=4)
        for j in range(8):
            t = ci * CH + h * 8 + j
            nc.tensor.transpose(ph[:, j * 128:(j + 1) * 128],
                                st.embbuf[:, t * D:(t + 1) * D], st.ident16)
        dst = ebT[:, h * H:(h + 1) * H]
        # GPSIMD cannot read PSUM: drains go ACT (half A) + DVE (half B);
        # c2/c3 go fully to ACT so the mini chain's rounds stay dense.
        if h == 0 or ci in (2, 3, 4):
            nc.scalar.copy(dst, ph)
        else:
            nc.vector.tensor_copy(dst, ph)
    st.ebts[ci] = ebT


def _score_matvec(st, ci):
    """Matvec a transposed chunk against w_col; copy score columns out."""
    _mark(st, f"scoreM_c{ci}")
    nc = st.nc
    ebT = st.ebts[ci]
    psc = st.psum.tile([128, CH], f32, tag="psc", name=f"psc{ci}", bufs=2)
    for j in range(CH):
        nc.tensor.matmul(psc[:, j:j + 1], ebT[:, j * 128:(j + 1) * 128],
                         st.w_col)
    nc.vector.tensor_copy(st.scores_tm[:, ci * CH:(ci + 1) * CH], psc)


def _score_chunk(st, pos):
    """Process the pos-th chunk in CHUNK_ORDER; matvec lags by MV_LAG."""
    _score_transpose(st, CHUNK_ORDER[pos], pos)
    if pos >= MV_LAG:
        _score_matvec(st, CHUNK_ORDER[pos - MV_LAG])
    if pos == NCH - 1:
        for p in range(NCH - MV_LAG, NCH):
            _score_matvec(st, CHUNK_ORDER[p])


# ------------------------------------------------------------- thresholds

def _prefix(st, rng_name, tr_hi):
    """Build masked row-major scores srm[lo:hi]: PE transposes the
    even/odd tile columns straight out of scores_tm (strided read), and
    the PSUM drain folds the mask via srm = (s + C0)*mask in one
    scalar_tensor_tensor per half."""
    _mark(st, f"prefix_{rng_name}")
    nc = st.nc
    lo, hi = RANGES[rng_name]
    w_lo = (lo // 32) * 32
    # deinterleave even/odd tile columns (DVE strided copies), then PE
    # transpose and fold the mask in the PSUM drain
    n = hi - w_lo
    src = st.scores_tm
    for h, name in ((0, "e"), (1, "o")):
        sh = bass.AP(tensor=src.tensor, offset=src.offset + 2 * w_lo + h,
                     ap=[list(src.ap[0]), [2, n]])
        de = st.singles.tile([128, R], f32, tag=f"de{h}", name=f"de{h}")
        nc.vector.tensor_copy(de[:, w_lo:hi], sh)
        ps = st.psum.tile([n, 128], f32, tag=PFX_TAG,
                          name=f"ps{name}{rng_name}",
                          bufs=4 if PFX_TAG == "ptile" else 2)
        nc.tensor.transpose(ps, de[:, w_lo:hi], st.ident)
        nc.vector.scalar_tensor_tensor(
            out=st.srm[w_lo:hi, h * 128:(h + 1) * 128], in0=ps[0:n, :],
            scalar=C0, in1=st.mfs[w_lo:hi, h * 128:(h + 1) * 128],
            op0=Alu.add, op1=Alu.mult,
        )


ROUNDS_PRIO = False
C_SPLIT = False
CHUNK_ORDER = [0, 1, 2, 3, 4, 5, 6, 7]
PFX_TAG = "psc"
MV_LAG = 2
CHAIN_MODE = "merged"
EBT67_3WAY = False


def _rounds(st, rng_name, upto):
    """max8 + match_replace rounds on srm/work partition slice."""
    if ROUNDS_PRIO:
        with st.tc.high_priority():
            return _rounds_inner(st, rng_name, upto)
    return _rounds_inner(st, rng_name, upto)


def _rounds_inner(st, rng_name, upto):
    _mark(st, f"rounds_{rng_name}_{upto}")
    nc = st.nc
    lo, hi = RANGES[rng_name]
    rg = hi - lo
    done = st.rounds_done[lo:hi]
    assert (done == done[0]).all(), (rng_name, done)
    r0 = int(done[0])
    for r in range(r0, upto):
        if r not in st.mxr:
            st.mxr[r] = st.singles.tile([R, 8], f32, tag=f"mx_{r}",
                                        name=f"mx_{r}")
        mx = st.mxr[r][lo:hi, :]
        src = st.srm[lo:hi, :] if r == 0 else st.work[lo:hi, :]
        nc.vector.max(out=mx, in_=src)
        if r < 15:
            nc.vector.match_replace(out=st.work[lo:hi, :], in_to_replace=mx,
                                    in_values=src, imm_value=REPL)
    st.rounds_done[lo:hi] = upto
    return st.mxr[upto - 1][lo:hi, 7:8]


# ------------------------------------------------------------------ gates

GCOLS_PRIO = False


def _gcols(st, rng_name, k, thr, pair_eng="act", act_cols=False):
    """thr -> bias col -> sigmoid (f16) -> PE transpose -> paired gate
    cols gpair[(k, rng)] [128, 4*rg] (col 4j+2h+b for row lo+j, half h).
    act_cols additionally emits f32 columns (tile-local index) for
    ACT-assisted gating (Activation scale must be FP32)."""
    if GCOLS_PRIO:
        with st.tc.high_priority():
            _gcols_inner(st, rng_name, k, thr, pair_eng, act_cols)
    else:
        _gcols_inner(st, rng_name, k, thr, pair_eng, act_cols)


def _gcols_inner(st, rng_name, k, thr, pair_eng, act_cols=False):
    _mark(st, f"gcols_{rng_name}_{k}")
    nc = st.nc
    lo, hi = RANGES[rng_name]
    rg = hi - lo
    # dif = srm - thr on Pool (avoids the huge-offset cancellation the
    # folded bias would hit in ACT's scale/bias pipeline)
    if k not in st.dif:
        st.dif[k] = st.singles.tile([R, T], f32, tag=f"dif{k}",
                                    name=f"dif{k}")
    dif = st.dif[k][lo:hi, :]
    nc.gpsimd.tensor_scalar(out=dif, in0=st.srm[lo:hi, :], scalar1=thr,
                            scalar2=None, op0=Alu.subtract)
    if k not in st.sig:
        st.sig[k] = st.singles.tile([R, T], bf16, tag=f"sig{k}",
                                    name=f"sig{k}")
    sig = st.sig[k][lo:hi, :]
    nc.scalar.activation(sig, dif, Act.Sigmoid, bias=0.0,
                         scale=st.temp_col[lo:hi, :])
    g = st.singles.tile([128, 4 * rg], bf16, tag=f"g{rng_name}_{k}",
                        name=f"g{rng_name}_{k}")
    st.gpair[(k, rng_name)] = g
    g32 = None
    if act_cols:
        g32 = st.singles.tile([128, 2 * rg], f32, tag=f"g32{rng_name}_{k}",
                              name=f"g32{rng_name}_{k}")
        st.g32[(k, rng_name)] = g32
    cp = nc.scalar.copy if pair_eng == "act" else nc.vector.tensor_copy
    # PE needs 0/32/64-aligned base partitions: transpose a 32-aligned
    # window of the full-height sig tile, then slice the wanted rows.
    w_lo = 0 if lo < 32 else 32
    w_hi = 64 if hi > w_lo + 32 or rg > 32 else w_lo + 32
    w = w_hi - w_lo
    for h in range(2):
        pm = st.psum.tile([128, w], bf16, tag="pst",
                          name=f"pm{rng_name}{k}{h}", bufs=2)
        nc.tensor.transpose(pm, st.sig[k][w_lo:w_hi, h * 128:(h + 1) * 128],
                            st.identb[w_lo:w_hi, w_lo:w_hi])
        dst = bass.AP(tensor=g.tensor, offset=g.offset + 2 * h,
                      ap=[list(g.ap[0]), [4, rg], [1, 2]])
        src = bass.AP(tensor=pm.tensor, offset=pm.offset + (lo - w_lo),
                      ap=[list(pm.ap[0]), [1, rg], [0, 2]])
        cp(dst, src)
        if act_cols:
            d32 = bass.AP(tensor=g32.tensor, offset=g32.offset + h,
                          ap=[list(g32.ap[0]), [2, rg]])
            s32 = bass.AP(tensor=pm.tensor, offset=pm.offset + (lo - w_lo),
                          ap=[list(pm.ap[0]), [1, rg]])
            nc.scalar.copy(d32, s32)


def _gate(st, k, unit, eng, g_lo=0, n_g=4, store=True):
    """Gate groups [g_lo, g_lo+n_g) (8 tiles each) of unit on an engine.

    eng: 'dve' (one wide 2x TT), 'pool' (one TT per group), 'act'
    (per-tile activation copies).  store=True emits the group stores
    immediately after."""
    _mark(st, f"gate_{k}_{unit}_{eng}{g_lo}")
    nc = st.nc
    cand = ("F", "AB" if unit < 2 else "CC", UNIT_RANGE[unit])
    for rng_name in cand:
        if (k, rng_name) in st.gpair:
            break
    lo, hi = RANGES[rng_name]
    g = st.gpair[(k, rng_name)]
    och = st.och[(k, unit)]
    t0 = unit * UNIT_TILES
    if eng == "dve":
        # two-group (16-tile) ops: small enough to fill chain holes
        gis = range(g_lo, g_lo + n_g, 2)
    else:
        gis = range(g_lo, g_lo + n_g)
    for gi in gis:
        ts = t0 + gi * 8
        n = 16 if eng == "dve" else 8
        col0 = 2 * (ts - 2 * lo)
        gb = bass.AP(tensor=g.tensor, offset=g.offset + col0,
                     ap=[list(g.ap[0]), [2, n], [0, 64], [1, 2]])
        o_ap = och[:, (ts - t0) * D:(ts - t0 + n) * D]
        e_ap = st.embbuf[:, ts * D:(ts + n) * D]
        if eng == "dve":
            nc.vector.tensor_tensor(o_ap, e_ap, gb, op=Alu.mult)
        elif eng == "pool":
            nc.gpsimd.tensor_tensor(o_ap, e_ap, gb, op=Alu.mult)
        else:  # act: per-tile activation with f32 scale col
            g32 = st.g32[(k, rng_name)]
            for j in range(8):
                t = ts + j
                scol = g32[:, (t - 2 * lo):(t - 2 * lo) + 1]
                nc.scalar.activation(
                    och[:, (t - t0) * D:(t - t0 + 1) * D],
                    st.embbuf[:, t * D:(t + 1) * D], Act.Copy,
                    bias=0.0, scale=scol)
        if store:
            _store(st, k, unit, gi, 2 if eng == "dve" else 1)


def _store(st, k, unit, g_lo=0, n_g=4):
    _mark(st, f"store_{k}_{unit}_{g_lo}")
    nc = st.nc
    k_i = KS.index(k)
    t0 = unit * UNIT_TILES + g_lo * 8
    n = n_g * 8
    och = st.och[(k, unit)]
    nc.sync.dma_start(
        out=st.out.ap()[k_i, :, t0 * D:(t0 + n) * D],
        in_=och[:, (t0 - unit * UNIT_TILES) * D:
                (t0 - unit * UNIT_TILES + n) * D],
    )


def _new_och(st, k, unit):
    st.och[(k, unit)] = st.opool.tile([128, UNIT_TILES * D], bf16, tag="och",
                                      name=f"och{k}_{unit}")


def _mini(st, mname, row_lo):
    """Early k=32 chain for 16 rows on dedicated partition-0 tiles.

    Rows [row_lo, row_lo+16) can't be partition-sliced out of the shared
    srm (engine ops need 32-aligned starts), but a dedicated tile starts
    at partition 0, so the chain can run the moment its two chunks are
    scored — unit (32, row_lo//16) stores while later chunks still load."""
    _mark(st, f"mini_{mname}")
    nc = st.nc
    rg = 16
    srmx = st.singles.tile([rg, T], f32, tag=f"srm{mname}", name=f"srm{mname}")
    workx = st.singles.tile([rg, T], f32, tag=f"work{mname}",
                            name=f"work{mname}")
    mfx = st.mfs_mini[mname]
    src = st.scores_tm
    for h in range(2):
        sh = bass.AP(tensor=src.tensor, offset=src.offset + 2 * row_lo + h,
                     ap=[list(src.ap[0]), [2, rg]])
        de = st.singles.tile([128, rg], f32, tag=f"dem{mname}{h}",
                             name=f"dem{mname}{h}")
        nc.vector.tensor_copy(de, sh)
        ps = st.psum.tile([rg, 128], f32, tag="psc", name=f"psm{mname}{h}",
                          bufs=2)
        nc.tensor.transpose(ps, de, st.ident)
        nc.vector.scalar_tensor_tensor(
            out=srmx[:, h * 128:(h + 1) * 128], in0=ps,
            scalar=C0, in1=mfx[:, h * 128:(h + 1) * 128],
            op0=Alu.add, op1=Alu.mult,
        )
    for r in range(4):
        mx = st.singles.tile([rg, 8], f32, tag=f"mxm{mname}{r}",
                             name=f"mxm{mname}{r}")
        s0 = srmx if r == 0 else workx
        nc.vector.max(out=mx, in_=s0)
        nc.vector.match_replace(out=workx, in_to_replace=mx, in_values=s0,
                                imm_value=REPL)
        thr = mx[:, 7:8]
    dif = st.singles.tile([rg, T], f32, tag=f"difm{mname}",
                          name=f"difm{mname}")
    nc.gpsimd.tensor_scalar(out=dif, in0=srmx, scalar1=thr, scalar2=None,
                            op0=Alu.subtract)
    sig = st.singles.tile([rg, T], bf16, tag=f"sigm{mname}",
                          name=f"sigm{mname}")
    nc.scalar.activation(sig, dif, Act.Sigmoid, bias=0.0,
                         scale=st.temp_col[0:rg, :])
    g = st.singles.tile([128, 4 * rg], bf16, tag=f"gm{mname}",
                        name=f"gm{mname}")
    st.gpair[(32, mname)] = g
    for h in range(2):
        pm = st.psum.tile([128, rg], bf16, tag="pst", name=f"pmm{mname}{h}",
                          bufs=2)
        nc.tensor.transpose(pm, sig[:, h * 128:(h + 1) * 128],
                            st.identb[0:rg, 0:rg])
        dst = bass.AP(tensor=g.tensor, offset=g.offset + 2 * h,
                      ap=[list(g.ap[0]), [4, rg], [1, 2]])
        s2 = bass.AP(tensor=pm.tensor, offset=pm.offset,
                     ap=[list(pm.ap[0]), [1, rg], [0, 2]])
        nc.vector.tensor_copy(dst, s2)


# ------------------------------------------------------------------- emit

def _emit(st, nc, emb, w, temp, mask, ident_d):
    singles = st.singles
    st.ebts = {}
    st.mxr = {}
    st.sig = {}
    st.dif = {}
    st.g32 = {}
    st.gpair = {}
    st.och = {}
    st.rounds_done = np.zeros(R, dtype=int)

    # ---- loads (sync queue order == service order) ----
    embbuf = singles.tile([128, NT * D], EMB_DT)
    st.embbuf = embbuf

    def load_chunk(ci):
        nc.sync.dma_start(
            out=embbuf[:, ci * CH * D:(ci + 1) * CH * D],
            in_=emb.ap()[:, ci * CH * D:(ci + 1) * CH * D])

    # PE p-state warmup on a memset tile (runs before any DMA lands)
    wtile = singles.tile([128, 128], f32)
    nc.gpsimd.memset(wtile, 0.0)
    pwarm = st.psum.tile([128, 128], f32, tag="pst", name="pwarm", bufs=2)
    for _ in range(10):
        nc.tensor.transpose(pwarm, wtile, wtile)

    ident = singles.tile([128, 128], f32)
    nc.sync.dma_start(out=ident, in_=ident_d.ap())
    st.ident = ident
    load_chunk(CHUNK_ORDER[0])
    load_chunk(CHUNK_ORDER[1])
    w_col_stage = singles.tile([128, 1], f32)
    nc.sync.dma_start(
        out=w_col_stage,
        in_=bass.AP(tensor=w.ap().tensor, offset=0, ap=[[1, 128], [0, 1]]))
    mask_sb = singles.tile([R, T], u8)
    nc.sync.dma_start(out=mask_sb, in_=mask.ap())
    st.mask_sb = mask_sb
    mask_b = singles.tile([16, T], u8)
    nc.sync.dma_start(out=mask_b, in_=mask.ap()[16:32, :])
    st.mask_b = mask_b
    tc_stage = singles.tile([R, 1], f32)
    nc.sync.dma_start(out=tc_stage, in_=_bcast(temp.ap(), R))
    for p in range(2, NCH):
        load_chunk(CHUNK_ORDER[p])

    # ---- SBUF constants (DVE: keep ACT/Pool queues free for ebT) ----
    sig_warm = singles.tile([1, 1], f32)
    nc.scalar.activation(sig_warm, ident[0:1, 0:1], Act.Sigmoid,
                         bias=0.0, scale=1.0)
    ident16 = singles.tile([128, 128], EMB_DT)
    nc.vector.tensor_copy(ident16, ident)
    st.ident16 = ident16
    identb = singles.tile([128, 128], bf16)
    nc.vector.tensor_copy(identb, ident)
    st.identb = identb
    w_col = singles.tile([128, 1], EMB_DT)
    nc.vector.tensor_copy(w_col, w_col_stage)
    st.w_col = w_col
    temp_col = singles.tile([R, 1], f32)
    nc.vector.tensor_scalar_mul(temp_col, tc_stage, 1.0 / EMB_SCALE)
    st.temp_col = temp_col
    negtemp_col = singles.tile([R, 1], f32)
    nc.vector.tensor_scalar_mul(negtemp_col, tc_stage, -1.0 / EMB_SCALE)
    st.negtemp_col = negtemp_col

    st.scores_tm = singles.tile([128, NT], f32)
    nc.vector.memset(st.scores_tm, 0.0)
    st.srm = singles.tile([R, T], f32)
    st.work = singles.tile([R, T], f32)
    mfs = singles.tile([R, T], f32)
    nc.vector.tensor_copy(mfs, mask_sb)
    st.mfs = mfs
    mfs_b = singles.tile([16, T], f32)
    nc.vector.tensor_copy(mfs_b, mask_b)
    st.mfs_mini = {"A": mfs[0:16, :], "B": mfs_b}

    # ---- pipeline ----
    _score_transpose(st, 0, 0)
    _score_transpose(st, 1, 1)
    _score_matvec(st, 0)
    _score_matvec(st, 1)
    _mini(st, "A", 0)
    _new_och(st, 32, 0)
    _gate(st, 32, 0, "dve")
    _score_transpose(st, 2, 2)
    _score_transpose(st, 3, 3)
    _score_matvec(st, 2)
    _score_matvec(st, 3)
    _prefix(st, "AB", 32)
    thrAB32 = _rounds(st, "AB", 4)
    _gcols(st, "AB", 32, thrAB32, pair_eng="dve", act_cols=True)
    _new_och(st, 32, 1)
    _gate(st, 32, 1, "pool", g_lo=0, n_g=2, store=True)
    _gate(st, 32, 1, "act", g_lo=2, n_g=2, store=True)
    _score_transpose(st, 4, 4)
    _score_transpose(st, 5, 5)
    _score_matvec(st, 4)
    _score_matvec(st, 5)
    _score_transpose(st, 6, 6)
    _score_transpose(st, 7, 7)
    _score_matvec(st, 6)
    _score_matvec(st, 7)
    _prefix(st, "CC", 64)
    thrC32 = _rounds(st, "CC", 4)
    _gcols(st, "CC", 32, thrC32)
    _new_och(st, 32, 2)
    _gate(st, 32, 2, "dve")
    _new_och(st, 32, 3)
    _gate(st, 32, 3, "dve")
    thrAB64 = _rounds(st, "AB", 8)
    _gcols(st, "AB", 64, thrAB64, act_cols=True)
    _new_och(st, 64, 0)
    _gate(st, 64, 0, "dve")
    # (64,1): Pool g0,g1 + ACT g2,g3 while DVE runs the CC chain
    _new_och(st, 64, 1)
    _gate(st, 64, 1, "pool", g_lo=0, n_g=2, store=True)
    _gate(st, 64, 1, "act", g_lo=2, n_g=2, store=True)
    thrC64 = _rounds(st, "CC", 8)
    _gcols(st, "CC", 64, thrC64, act_cols=True)
    _new_och(st, 64, 2)
    _gate(st, 64, 2, "pool", g_lo=0, n_g=2, store=True)
    _gate(st, 64, 2, "act", g_lo=2, n_g=2, store=True)
    _new_och(st, 64, 3)
    _gate(st, 64, 3, "dve")
    thr128 = _rounds(st, "F", 16)
    _gcols(st, "F", 128, thr128, act_cols=True)
    _new_och(st, 128, 0)
    _gate(st, 128, 0, "dve")
    _new_och(st, 128, 1)
    _gate(st, 128, 1, "pool", g_lo=0, n_g=2, store=True)
    _gate(st, 128, 1, "act", g_lo=2, n_g=2, store=True)
    _new_och(st, 128, 2)
    _gate(st, 128, 2, "dve")
    _new_och(st, 128, 3)
    _gate(st, 128, 3, "dve")


# -------------------------------------------------------------- host glue

_NC = None


def _get_nc():
    global _NC
    if _NC is None:
        _NC = build_bass()
    return _NC


def make_in_maps(embeddings, w, temperature, mask):
    emb = np.asarray(embeddings, dtype=np.float32)
    w = np.ascontiguousarray(np.asarray(w, dtype=np.float32))
    temp = np.ascontiguousarray(np.asarray(temperature, dtype=np.float32))
    mask_u8 = np.asarray(mask).astype(np.uint8)
    in_maps = []
    for c in range(N_CORES):
        sl = slice(c * R, (c + 1) * R)
        esh = emb[sl].reshape(NT, 128, D).transpose(1, 0, 2).reshape(128, NT * D)
        esh = esh * EMB_SCALE
        in_maps.append({
            "emb_tm": np.ascontiguousarray(esh.astype(EMB_NP)),
            "w": w,
            "temperature": temp,
            "mask": np.ascontiguousarray(mask_u8[sl]),
        })
    return in_maps


def postprocess(results):
    outs = []
    for r in results:
        o = np.asarray(r["out"]).astype(np.float32)
        o *= 1.0 / EMB_SCALE
        o = o.reshape(len(KS), 128, NT, D).transpose(0, 2, 1, 3)
        outs.append(o.reshape(len(KS), R, T, D))
    return np.concatenate(outs, axis=1)


def kernel(embeddings, w, b, temperature, mask):
    nc = _get_nc()
    in_maps = make_in_maps(embeddings, w, temperature, mask)
    res = run_bass_kernel_spmd(nc, in_maps, core_ids=list(range(N_CORES)))
    return postprocess(res.results)


# revision 5
# speedup vs baseline: 1.0204x; 1.0094x over previous
"""Matryoshka soft-top-k gating kernel for Trainium2 (Bass/Tile) — v2.1.

Computes, for each matryoshka scale k in (128, 64, 32):
    scores  = emb @ w  (+ b, which cancels in scores - threshold)
    srm     = (scores + C0) * mask      (order-preserving shift: masked
              positions sit at 0, far below every real score near C0)
    thr_k   = k-th largest srm per row (max8 + match_replace chains)
    gate    = sigmoid((srm - thr_k) * temperature)  (masked -> ~0)
    out_k   = emb * gate[..., None]

Sharding: data-parallel over the batch axis across 8 NeuronCores
(64 rows per core); w/temperature replicated, mask sharded with batch.

HBM traffic is the roofline (16 MiB/core at ~360 B/ns => ~47 us), so
embeddings upload as fp16 pre-scaled by 2^10 and outputs store as bf16;
the host pre-transposes embeddings into token-major [128, NT*D] and
reassembles/unscales the outputs.

Engine plan (78.3 us -> ~67 us in the TimelineSim cost model):
  - Gating runs as wide DVE tensor_tensor ops in 2x mode: the gate
    operand is materialized as bf16 PAIRS (each gate duplicated, AP
    innermost [1,2]) so every operand is 2-byte packed -> ~69 ns/tile
    instead of 1x broadcast TT (133), Pool TT (266) or ACT act-copy
    (330).  bf16 (not fp16) because near-zero gates underflow fp16's
    6e-8 floor, which shows up against the harness' 1e-6 rel floor.
    Pool and ACT co-gate four units (g0/g1 on Pool TT, g2/g3 as ACT
    per-tile activations with f32 scale columns).
  - A dedicated "mini" chain for rows 0-15 on its own partition-0
    tiles starts the moment chunks c0/c1 are scored, so store unit
    (32,0) is ready roughly when the input load drains.  Engine ops
    need 32-aligned partition starts, so 16-row slices of the shared
    srm are illegal — a separate tile sidesteps that.
  - One shared row-major score tile srm/work [64,256] with chains on
    32-aligned slices (AB rows 0-31, CC rows 32-63, F all): AB 1-8,
    CC 1-8, F 9-16 supply every remaining threshold.
  - The mask folds into the PSUM drain of the score transposes as
    srm = (s + C0)*mask (one scalar_tensor_tensor per half); the
    explicit dif = srm - thr stays a separate Pool op because folding
    thr into the sigmoid bias cancels catastrophically in ACT's
    scale/bias path.
  - ebT (transposed-chunk) drains split ACT (half A) + DVE (half B),
    with c2-c4 fully on ACT so the mini/AB chain rounds stay dense;
    GPSIMD cannot touch PSUM, so Pool stays out of the score pipeline
    and spends its time on gating instead.
  - Stores are emitted in producer order, 8/16-tile granularity, so the
    serialized DMA engines restart as soon as each group is gated.
"""

import numpy as np

import concourse.bacc as bacc
import concourse.bass as bass
import concourse.mybir as mybir
import concourse.tile as tile
from concourse.bass_utils import run_bass_kernel_spmd

N_CORES = 8
B, T, D = 512, 256, 128
R = B // N_CORES          # rows (documents) per core
KS = (128, 64, 32)
# Masked fill via order-preserving shift: srm = (s + C0)*mask.  Unmasked
# scores land near C0 (+-6e3), masked at 0 (far below), and the uniform
# +C0 shift cancels in s - thr inside the sigmoid bias.  C0 is small
# enough that f32 keeps ~0.008 absolute score precision (scores are
# pre-scaled by 2^10, so that is ~8e-6 in model units).
C0 = 1.0e5
REPL = -1.0               # match_replace sentinel (below masked 0)
NT = R * T // 128         # 128-token tiles per core (128)
CH = 16                   # tiles per DMA chunk
NCH = NT // CH            # 8 chunks
UNIT_TILES = 32           # tiles per store unit (1 MiB)
N_UNITS = 4

EMB_DT = mybir.dt.float16
EMB_NP = np.float16
EMB_SCALE = 1024.0

f32 = mybir.dt.float32
f16 = mybir.dt.float16
bf16 = mybir.dt.bfloat16
u8 = mybir.dt.uint8
Alu = mybir.AluOpType
Act = mybir.ActivationFunctionType

# row ranges (engine partition starts must be 32-aligned; the 16-row
# A/B ranges live on their own partition-0-based mini tiles)
RANGES = {"A": (0, 16), "B": (16, 32), "AB": (0, 32), "CC": (32, 64),
          "F": (0, 64)}
# unit -> range owning its rows
UNIT_RANGE = {0: "A", 1: "B", 2: "CC", 3: "CC"}
# ebT split: partner engine takes the leading cols (ready after the
# first transposes), ACT the tail.  c0-c3 partner with DVE (fills DVE's
# idle front-end), c4-c7 with Pool.
EBT_SPLIT = {"dve": 768, "pool": 768}

PHASES = []


class _St:
    pass


def _bcast(ap, n):
    return bass.AP(tensor=ap.tensor, offset=ap.offset, ap=[[0, n]] + list(ap.ap))


def build_bass():
    nc = bacc.Bacc("TRN2", target_bir_lowering=False, debug=False)

    emb = nc.dram_tensor("emb_tm", [128, NT * D], EMB_DT, kind="ExternalInput")
    w = nc.dram_tensor("w", [D], f32, kind="ExternalInput")
    temp = nc.dram_tensor("temperature", [1], f32, kind="ExternalInput")
    mask = nc.dram_tensor("mask", [R, T], u8, kind="ExternalInput")
    out = nc.dram_tensor("out", [len(KS), 128, NT * D], bf16,
                         kind="ExternalOutput")
    ident_d = nc.inline_tensor(np.eye(128, dtype=np.float32), name="ident128")

    st = _St()
    st.nc = nc
    st.out = out

    with tile.TileContext(nc) as tc:
        with (
            tc.tile_pool(name="singles", bufs=1) as singles,
            tc.tile_pool(name="out0", bufs=8) as opool,
            tc.tile_pool(name="ebt", bufs=3) as ebtpool,
            tc.tile_pool(name="psum", bufs=2, space="PSUM") as psum,
        ):
            st.singles = singles
            st.opool = opool
            st.ebtpool = ebtpool
            st.psum = psum
            st.tc = tc
            _emit(st, nc, emb, w, temp, mask, ident_d)

    nc.compile()
    return nc


def _mark(st, label):
    PHASES.append((label, st.nc.next_id()))


# ---------------------------------------------------------------- scoring

def _score_transpose(st, ci, pos):
    """PE-transpose the chunk in two 8-tile halves, each to its own
    1-bank PSUM tile (bufs=4), drained by alternating engines so the
    transpose->SBUF pipeline keeps up with the load cadence."""
    _mark(st, f"scoreT_c{ci}")
    nc = st.nc
    ebT = st.ebtpool.tile([128, CH * 128], EMB_DT, tag="ebT")
    H = CH * 128 // 2
    for h in range(2):
        ph = st.psum.tile([128, H], EMB_DT, tag="ptile",
                          name=f"pt{ci}_{h}", bufs=4)
        for j in range(8):
            t = ci * CH + h * 8 + j
            nc.tensor.transpose(ph[:, j * 128:(j + 1) * 128],
                                st.embbuf[:, t * D:(t + 1) * D], st.ident16)
        dst = ebT[:, h * H:(h + 1) * H]
        # GPSIMD cannot read PSUM: drains go ACT (half A) + DVE (half B);
        # c2/c3 go fully to ACT so the mini chain's rounds stay dense.
        if h == 0 or ci in (2, 3, 4):
            nc.scalar.copy(dst, ph)
        else:
            nc.vector.tensor_copy(dst, ph)
    st.ebts[ci] = ebT


def _score_matvec(st, ci):
    """Matvec a transposed chunk against w_col; copy score columns out."""
    _mark(st, f"scoreM_c{ci}")
    nc = st.nc
    ebT = st.ebts[ci]
    psc = st.psum.tile([128, CH], f32, tag="psc", name=f"psc{ci}", bufs=2)
    for j in range(CH):
        nc.tensor.matmul(psc[:, j:j + 1], ebT[:, j * 128:(j + 1) * 128],
                         st.w_col)
    nc.vector.tensor_copy(st.scores_tm[:, ci * CH:(ci + 1) * CH], psc)


def _score_chunk(st, pos):
    """Process the pos-th chunk in CHUNK_ORDER; matvec lags by MV_LAG."""
    _score_transpose(st, CHUNK_ORDER[pos], pos)
    if pos >= MV_LAG:
        _score_matvec(st, CHUNK_ORDER[pos - MV_LAG])
    if pos == NCH - 1:
        for p in range(NCH - MV_LAG, NCH):
            _score_matvec(st, CHUNK_ORDER[p])


# ------------------------------------------------------------- thresholds

def _prefix(st, rng_name, tr_hi):
    """Build masked row-major scores srm[lo:hi]: PE transposes the
    even/odd tile columns straight out of scores_tm (strided read), and
    the PSUM drain folds the mask via srm = (s + C0)*mask in one
    scalar_tensor_tensor per half."""
    _mark(st, f"prefix_{rng_name}")
    nc = st.nc
    lo, hi = RANGES[rng_name]
    w_lo = (lo // 32) * 32
    # deinterleave even/odd tile columns (DVE strided copies), then PE
    # transpose and fold the mask in the PSUM drain
    n = hi - w_lo
    src = st.scores_tm
    for h, name in ((0, "e"), (1, "o")):
        sh = bass.AP(tensor=src.tensor, offset=src.offset + 2 * w_lo + h,
                     ap=[list(src.ap[0]), [2, n]])
        de = st.singles.tile([128, R], f32, tag=f"de{h}", name=f"de{h}")
        nc.vector.tensor_copy(de[:, w_lo:hi], sh)
        ps = st.psum.tile([n, 128], f32, tag=PFX_TAG,
                          name=f"ps{name}{rng_name}",
                          bufs=4 if PFX_TAG == "ptile" else 2)
        nc.tensor.transpose(ps, de[:, w_lo:hi], st.ident)
        nc.vector.scalar_tensor_tensor(
            out=st.srm[w_lo:hi, h * 128:(h + 1) * 128], in0=ps[0:n, :],
            scalar=C0, in1=st.mfs[w_lo:hi, h * 128:(h + 1) * 128],
            op0=Alu.add, op1=Alu.mult,
        )


ROUNDS_PRIO = False
C_SPLIT = False
CHUNK_ORDER = [0, 1, 2, 3, 4, 5, 6, 7]
PFX_TAG = "psc"
MV_LAG = 2
CHAIN_MODE = "merged"
EBT67_3WAY = False


def _rounds(st, rng_name, upto):
    """max8 + match_replace rounds on srm/work partition slice."""
    if ROUNDS_PRIO:
        with st.tc.high_priority():
            return _rounds_inner(st, rng_name, upto)
    return _rounds_inner(st, rng_name, upto)


def _rounds_inner(st, rng_name, upto):
    _mark(st, f"rounds_{rng_name}_{upto}")
    nc = st.nc
    lo, hi = RANGES[rng_name]
    rg = hi - lo
    done = st.rounds_done[lo:hi]
    assert (done == done[0]).all(), (rng_name, done)
    r0 = int(done[0])
    for r in range(r0, upto):
        if r not in st.mxr:
            st.mxr[r] = st.singles.tile([R, 8], f32, tag=f"mx_{r}",
                                        name=f"mx_{r}")
        mx = st.mxr[r][lo:hi, :]
        src = st.srm[lo:hi, :] if r == 0 else st.work[lo:hi, :]
        nc.vector.max(out=mx, in_=src)
        if r < 15:
            nc.vector.match_replace(out=st.work[lo:hi, :], in_to_replace=mx,
                                    in_values=src, imm_value=REPL)
    st.rounds_done[lo:hi] = upto
    return st.mxr[upto - 1][lo:hi, 7:8]


# ------------------------------------------------------------------ gates

GCOLS_PRIO = False


def _gcols(st, rng_name, k, thr, pair_eng="act", act_cols=False):
    """thr -> bias col -> sigmoid (f16) -> PE transpose -> paired gate
    cols gpair[(k, rng)] [128, 4*rg] (col 4j+2h+b for row lo+j, half h).
    act_cols additionally emits f32 columns (tile-local index) for
    ACT-assisted gating (Activation scale must be FP32)."""
    if GCOLS_PRIO:
        with st.tc.high_priority():
            _gcols_inner(st, rng_name, k, thr, pair_eng, act_cols)
    else:
        _gcols_inner(st, rng_name, k, thr, pair_eng, act_cols)


def _gcols_inner(st, rng_name, k, thr, pair_eng, act_cols=False):
    _mark(st, f"gcols_{rng_name}_{k}")
    nc = st.nc
    lo, hi = RANGES[rng_name]
    rg = hi - lo
    # dif = srm - thr on Pool (avoids the huge-offset cancellation the
    # folded bias would hit in ACT's scale/bias pipeline)
    if k not in st.dif:
        st.dif[k] = st.singles.tile([R, T], f32, tag=f"dif{k}",
                                    name=f"dif{k}")
    dif = st.dif[k][lo:hi, :]
    nc.gpsimd.tensor_scalar(out=dif, in0=st.srm[lo:hi, :], scalar1=thr,
                            scalar2=None, op0=Alu.subtract)
    if k not in st.sig:
        st.sig[k] = st.singles.tile([R, T], bf16, tag=f"sig{k}",
                                    name=f"sig{k}")
    sig = st.sig[k][lo:hi, :]
    nc.scalar.activation(sig, dif, Act.Sigmoid, bias=0.0,
                         scale=st.temp_col[lo:hi, :])
    g = st.singles.tile([128, 4 * rg], bf16, tag=f"g{rng_name}_{k}",
                        name=f"g{rng_name}_{k}")
    st.gpair[(k, rng_name)] = g
    g32 = None
    if act_cols:
        g32 = st.singles.tile([128, 2 * rg], f32, tag=f"g32{rng_name}_{k}",
                              name=f"g32{rng_name}_{k}")
        st.g32[(k, rng_name)] = g32
    cp = nc.scalar.copy if pair_eng == "act" else nc.vector.tensor_copy
    # PE needs 0/32/64-aligned base partitions: transpose a 32-aligned
    # window of the full-height sig tile, then slice the wanted rows.
    w_lo = 0 if lo < 32 else 32
    w_hi = 64 if hi > w_lo + 32 or rg > 32 else w_lo + 32
    w = w_hi - w_lo
    for h in range(2):
        pm = st.psum.tile([128, w], bf16, tag="pst",
                          name=f"pm{rng_name}{k}{h}", bufs=2)
        nc.tensor.transpose(pm, st.sig[k][w_lo:w_hi, h * 128:(h + 1) * 128],
                            st.identb[w_lo:w_hi, w_lo:w_hi])
        dst = bass.AP(tensor=g.tensor, offset=g.offset + 2 * h,
                      ap=[list(g.ap[0]), [4, rg], [1, 2]])
        src = bass.AP(tensor=pm.tensor, offset=pm.offset + (lo - w_lo),
                      ap=[list(pm.ap[0]), [1, rg], [0, 2]])
        cp(dst, src)
        if act_cols:
            d32 = bass.AP(tensor=g32.tensor, offset=g32.offset + h,
                          ap=[list(g32.ap[0]), [2, rg]])
            s32 = bass.AP(tensor=pm.tensor, offset=pm.offset + (lo - w_lo),
                          ap=[list(pm.ap[0]), [1, rg]])
            nc.scalar.copy(d32, s32)


def _gate(st, k, unit, eng, g_lo=0, n_g=4, store=True):
    """Gate groups [g_lo, g_lo+n_g) (8 tiles each) of unit on an engine.

    eng: 'dve' (one wide 2x TT), 'pool' (one TT per group), 'act'
    (per-tile activation copies).  store=True emits the group stores
    immediately after."""
    _mark(st, f"gate_{k}_{unit}_{eng}{g_lo}")
    nc = st.nc
    cand = ("F", "AB" if unit < 2 else "CC", UNIT_RANGE[unit])
    for rng_name in cand:
        if (k, rng_name) in st.gpair:
            break
    lo, hi = RANGES[rng_name]
    g = st.gpair[(k, rng_name)]
    och = st.och[(k, unit)]
    t0 = unit * UNIT_TILES
    if eng == "dve":
        # two-group (16-tile) ops: small enough to fill chain holes
        gis = range(g_lo, g_lo + n_g, 2)
    else:
        gis = range(g_lo, g_lo + n_g)
    for gi in gis:
        ts = t0 + gi * 8
        n = 16 if eng == "dve" else 8
        col0 = 2 * (ts - 2 * lo)
        gb = bass.AP(tensor=g.tensor, offset=g.offset + col0,
                     ap=[list(g.ap[0]), [2, n], [0, 64], [1, 2]])
        o_ap = och[:, (ts - t0) * D:(ts - t0 + n) * D]
        e_ap = st.embbuf[:, ts * D:(ts + n) * D]
        if eng == "dve":
            nc.vector.tensor_tensor(o_ap, e_ap, gb, op=Alu.mult)
        elif eng == "pool":
            nc.gpsimd.tensor_tensor(o_ap, e_ap, gb, op=Alu.mult)
        else:  # act: per-tile activation with f32 scale col
            g32 = st.g32[(k, rng_name)]
            for j in range(8):
                t = ts + j
                scol = g32[:, (t - 2 * lo):(t - 2 * lo) + 1]
                nc.scalar.activation(
                    och[:, (t - t0) * D:(t - t0 + 1) * D],
                    st.embbuf[:, t * D:(t + 1) * D], Act.Copy,
                    bias=0.0, scale=scol)
        if store:
            _store(st, k, unit, gi, 2 if eng == "dve" else 1)


def _store(st, k, unit, g_lo=0, n_g=4):
    _mark(st, f"store_{k}_{unit}_{g_lo}")
    nc = st.nc
    k_i = KS.index(k)
    t0 = unit * UNIT_TILES + g_lo * 8
    n = n_g * 8
    och = st.och[(k, unit)]
    nc.sync.dma_start(
        out=st.out.ap()[k_i, :, t0 * D:(t0 + n) * D],
        in_=och[:, (t0 - unit * UNIT_TILES) * D:
                (t0 - unit * UNIT_TILES + n) * D],
    )


def _new_och(st, k, unit):
    st.och[(k, unit)] = st.opool.tile([128, UNIT_TILES * D], bf16, tag="och",
                                      name=f"och{k}_{unit}")


def _mini(st, mname, row_lo):
    """Early k=32 chain for 16 rows on dedicated partition-0 tiles.

    Rows [row_lo, row_lo+16) can't be partition-sliced out of the shared
    srm (engine ops need 32-aligned starts), but a dedicated tile starts
    at partition 0, so the chain can run the moment its two chunks are
    scored — unit (32, row_lo//16) stores while later chunks still load."""
    _mark(st, f"mini_{mname}")
    nc = st.nc
    rg = 16
    srmx = st.singles.tile([rg, T], f32, tag=f"srm{mname}", name=f"srm{mname}")
    workx = st.singles.tile([rg, T], f32, tag=f"work{mname}",
                            name=f"work{mname}")
    mfx = st.mfs_mini[mname]
    src = st.scores_tm
    for h in range(2):
        sh = bass.AP(tensor=src.tensor, offset=src.offset + 2 * row_lo + h,
                     ap=[list(src.ap[0]), [2, rg]])
        de = st.singles.tile([128, rg], f32, tag=f"dem{mname}{h}",
                             name=f"dem{mname}{h}")
        nc.vector.tensor_copy(de, sh)
        ps = st.psum.tile([rg, 128], f32, tag="psc", name=f"psm{mname}{h}",
                          bufs=2)
        nc.tensor.transpose(ps, de, st.ident)
        nc.vector.scalar_tensor_tensor(
            out=srmx[:, h * 128:(h + 1) * 128], in0=ps,
            scalar=C0, in1=mfx[:, h * 128:(h + 1) * 128],
            op0=Alu.add, op1=Alu.mult,
        )
    for r in range(4):
        mx = st.singles.tile([rg, 8], f32, tag=f"mxm{mname}{r}",
                             name=f"mxm{mname}{r}")
        s0 = srmx if r == 0 else workx
        nc.vector.max(out=mx, in_=s0)
        nc.vector.match_replace(out=workx, in_to_replace=mx, in_values=s0,
                                imm_value=REPL)
        thr = mx[:, 7:8]
    dif = st.singles.tile([rg, T], f32, tag=f"difm{mname}",
                          name=f"difm{mname}")
    nc.gpsimd.tensor_scalar(out=dif, in0=srmx, scalar1=thr, scalar2=None,
                            op0=Alu.subtract)
    sig = st.singles.tile([rg, T], bf16, tag=f"sigm{mname}",
                          name=f"sigm{mname}")
    nc.scalar.activation(sig, dif, Act.Sigmoid, bias=0.0,
                         scale=st.temp_col[0:rg, :])
    g = st.singles.tile([128, 4 * rg], bf16, tag=f"gm{mname}",
                        name=f"gm{mname}")
    st.gpair[(32, mname)] = g
    g32 = st.singles.tile([128, 2 * rg], f32, tag=f"g32m{mname}",
                          name=f"g32m{mname}")
    st.g32[(32, mname)] = g32
    for h in range(2):
        pm = st.psum.tile([128, rg], bf16, tag="pst", name=f"pmm{mname}{h}",
                          bufs=2)
        nc.tensor.transpose(pm, sig[:, h * 128:(h + 1) * 128],
                            st.identb[0:rg, 0:rg])
        dst = bass.AP(tensor=g.tensor, offset=g.offset + 2 * h,
                      ap=[list(g.ap[0]), [4, rg], [1, 2]])
        s2 = bass.AP(tensor=pm.tensor, offset=pm.offset,
                     ap=[list(pm.ap[0]), [1, rg], [0, 2]])
        nc.vector.tensor_copy(dst, s2)
        d32 = bass.AP(tensor=g32.tensor, offset=g32.offset + h,
                      ap=[list(g32.ap[0]), [2, rg]])
        s32 = bass.AP(tensor=pm.tensor, offset=pm.offset,
                      ap=[list(pm.ap[0]), [1, rg]])
        nc.scalar.copy(d32, s32)


# ------------------------------------------------------------------- emit

def _emit(st, nc, emb, w, temp, mask, ident_d):
    singles = st.singles
    st.ebts = {}
    st.mxr = {}
    st.sig = {}
    st.dif = {}
    st.g32 = {}
    st.gpair = {}
    st.och = {}
    st.rounds_done = np.zeros(R, dtype=int)

    # ---- loads (sync queue order == service order) ----
    embbuf = singles.tile([128, NT * D], EMB_DT)
    st.embbuf = embbuf

    def load_chunk(ci):
        nc.sync.dma_start(
            out=embbuf[:, ci * CH * D:(ci + 1) * CH * D],
            in_=emb.ap()[:, ci * CH * D:(ci + 1) * CH * D])

    # PE p-state warmup on a memset tile (runs before any DMA lands)
    wtile = singles.tile([128, 128], f32)
    nc.gpsimd.memset(wtile, 0.0)
    pwarm = st.psum.tile([128, 128], f32, tag="pst", name="pwarm", bufs=2)
    for _ in range(10):
        nc.tensor.transpose(pwarm, wtile, wtile)

    ident = singles.tile([128, 128], f32)
    nc.sync.dma_start(out=ident, in_=ident_d.ap())
    st.ident = ident
    load_chunk(CHUNK_ORDER[0])
    load_chunk(CHUNK_ORDER[1])
    w_col_stage = singles.tile([128, 1], f32)
    nc.sync.dma_start(
        out=w_col_stage,
        in_=bass.AP(tensor=w.ap().tensor, offset=0, ap=[[1, 128], [0, 1]]))
    mask_sb = singles.tile([R, T], u8)
    nc.sync.dma_start(out=mask_sb, in_=mask.ap())
    st.mask_sb = mask_sb
    mask_b = singles.tile([16, T], u8)
    nc.sync.dma_start(out=mask_b, in_=mask.ap()[16:32, :])
    st.mask_b = mask_b
    tc_stage = singles.tile([R, 1], f32)
    nc.sync.dma_start(out=tc_stage, in_=_bcast(temp.ap(), R))
    for p in range(2, NCH):
        load_chunk(CHUNK_ORDER[p])

    # ---- SBUF constants (DVE: keep ACT/Pool queues free for ebT) ----
    sig_warm = singles.tile([1, 1], f32)
    nc.scalar.activation(sig_warm, ident[0:1, 0:1], Act.Sigmoid,
                         bias=0.0, scale=1.0)
    ident16 = singles.tile([128, 128], EMB_DT)
    nc.vector.tensor_copy(ident16, ident)
    st.ident16 = ident16
    identb = singles.tile([128, 128], bf16)
    nc.vector.tensor_copy(identb, ident)
    st.identb = identb
    w_col = singles.tile([128, 1], EMB_DT)
    nc.vector.tensor_copy(w_col, w_col_stage)
    st.w_col = w_col
    temp_col = singles.tile([R, 1], f32)
    nc.vector.tensor_scalar_mul(temp_col, tc_stage, 1.0 / EMB_SCALE)
    st.temp_col = temp_col
    negtemp_col = singles.tile([R, 1], f32)
    nc.vector.tensor_scalar_mul(negtemp_col, tc_stage, -1.0 / EMB_SCALE)
    st.negtemp_col = negtemp_col

    st.scores_tm = singles.tile([128, NT], f32)
    nc.vector.memset(st.scores_tm, 0.0)
    st.srm = singles.tile([R, T], f32)
    st.work = singles.tile([R, T], f32)
    mfs = singles.tile([R, T], f32)
    nc.vector.tensor_copy(mfs, mask_sb)
    st.mfs = mfs
    mfs_b = singles.tile([16, T], f32)
    nc.vector.tensor_copy(mfs_b, mask_b)
    st.mfs_mini = {"A": mfs[0:16, :], "B": mfs_b}

    # ---- pipeline ----
    _score_transpose(st, 0, 0)
    _score_transpose(st, 1, 1)
    _score_matvec(st, 0)
    _score_matvec(st, 1)
    _mini(st, "A", 0)
    _new_och(st, 32, 0)
    _gate(st, 32, 0, "dve", g_lo=0, n_g=2)
    _gate(st, 32, 0, "pool", g_lo=2, n_g=2, store=True)
    _score_transpose(st, 2, 2)
    _score_transpose(st, 3, 3)
    _score_matvec(st, 2)
    _score_matvec(st, 3)
    _prefix(st, "AB", 32)
    thrAB32 = _rounds(st, "AB", 4)
    _gcols(st, "AB", 32, thrAB32, pair_eng="dve", act_cols=True)
    _new_och(st, 32, 1)
    _gate(st, 32, 1, "pool", g_lo=0, n_g=2, store=True)
    _gate(st, 32, 1, "act", g_lo=2, n_g=2, store=True)
    _score_transpose(st, 4, 4)
    _score_transpose(st, 5, 5)
    _score_matvec(st, 4)
    _score_matvec(st, 5)
    _score_transpose(st, 6, 6)
    _score_transpose(st, 7, 7)
    _score_matvec(st, 6)
    _score_matvec(st, 7)
    _prefix(st, "CC", 64)
    thrC32 = _rounds(st, "CC", 4)
    _gcols(st, "CC", 32, thrC32)
    _new_och(st, 32, 2)
    _gate(st, 32, 2, "dve")
    _new_och(st, 32, 3)
    _gate(st, 32, 3, "dve")
    thrAB64 = _rounds(st, "AB", 8)
    _gcols(st, "AB", 64, thrAB64, act_cols=True)
    _new_och(st, 64, 0)
    _gate(st, 64, 0, "dve")
    # (64,1): Pool g0,g1 + ACT g2,g3 while DVE runs the CC chain
    _new_och(st, 64, 1)
    _gate(st, 64, 1, "pool", g_lo=0, n_g=2, store=True)
    _gate(st, 64, 1, "act", g_lo=2, n_g=2, store=True)
    thrC64 = _rounds(st, "CC", 8)
    _gcols(st, "CC", 64, thrC64, act_cols=True)
    _new_och(st, 64, 2)
    _gate(st, 64, 2, "pool", g_lo=0, n_g=2, store=True)
    _gate(st, 64, 2, "act", g_lo=2, n_g=2, store=True)
    _new_och(st, 64, 3)
    _gate(st, 64, 3, "dve")
    thr128 = _rounds(st, "F", 16)
    _gcols(st, "F", 128, thr128, act_cols=True)
    _new_och(st, 128, 0)
    _gate(st, 128, 0, "dve")
    _new_och(st, 128, 1)
    _gate(st, 128, 1, "pool", g_lo=0, n_g=2, store=True)
    _gate(st, 128, 1, "act", g_lo=2, n_g=2, store=True)
    _new_och(st, 128, 2)
    _gate(st, 128, 2, "dve")
    _new_och(st, 128, 3)
    _gate(st, 128, 3, "dve")


# -------------------------------------------------------------- host glue

_NC = None


def _get_nc():
    global _NC
    if _NC is None:
        _NC = build_bass()
    return _NC


def make_in_maps(embeddings, w, temperature, mask):
    emb = np.asarray(embeddings, dtype=np.float32)
    w = np.ascontiguousarray(np.asarray(w, dtype=np.float32))
    temp = np.ascontiguousarray(np.asarray(temperature, dtype=np.float32))
    mask_u8 = np.asarray(mask).astype(np.uint8)
    in_maps = []
    for c in range(N_CORES):
        sl = slice(c * R, (c + 1) * R)
        esh = emb[sl].reshape(NT, 128, D).transpose(1, 0, 2).reshape(128, NT * D)
        esh = esh * EMB_SCALE
        in_maps.append({
            "emb_tm": np.ascontiguousarray(esh.astype(EMB_NP)),
            "w": w,
            "temperature": temp,
            "mask": np.ascontiguousarray(mask_u8[sl]),
        })
    return in_maps


def postprocess(results):
    outs = []
    for r in results:
        o = np.asarray(r["out"]).astype(np.float32)
        o *= 1.0 / EMB_SCALE
        o = o.reshape(len(KS), 128, NT, D).transpose(0, 2, 1, 3)
        outs.append(o.reshape(len(KS), R, T, D))
    return np.concatenate(outs, axis=1)


def kernel(embeddings, w, b, temperature, mask):
    nc = _get_nc()
    in_maps = make_in_maps(embeddings, w, temperature, mask)
    res = run_bass_kernel_spmd(nc, in_maps, core_ids=list(range(N_CORES)))
    return postprocess(res.results)
